# revision 28
# baseline (speedup 1.0000x reference)
"""Trainium2 Bass kernel for a 2-relation DGL-style GNN layer.

Math (see reference):
    h   = concat(drug_f @ drug_w, disease_f @ disease_w)        [N, 128]
    deg = bincount(rows); norm = clip(deg,1)^-0.5
    out = norm * segsum(  (norm * h)[cols], rows )

Distribution (8 NeuronCores, SPMD):
  - Nodes row-sharded: core c owns rows [c*6250, (c+1)*6250).  Cores 0-3 hold
    drug rows, cores 4-7 disease rows, so each core projects its shard with a
    single 128x128 weight (supplied per-core), scales it by the source norm,
    and quantizes to bf16 (y).  y rows are PARTITION-MAJOR (row = p*tiles + t
    for node t*128+p) so each y write is one contiguous descriptor per
    partition.
  - Three gather passes, partitioned by edge SOURCE:
      pass 0 (L): source in this core's own shard -> gathered from y_local,
        written right after projection (~22us).  No collective dependency,
        so its SWDGE descriptor generation and DMA transfers run during the
        ~66us CC-engine init + AllGather dead window.
      pass 1 (B): remote sources with src tile in [0,25) -> table0 (AG1).
      pass 2 (C): remote sources with src tile in [25,49) -> table1 (AG2).
    Tables are < 2^15 rows so int16 gather indices address them directly.
  - Edge slots are PACKED: per pass, tile t owns slot range
    [cum[t], cum[t]+M[t]) where M[t] = max over cores of the core's edge
    count for (pass, t) — no rounding of each tile to 128.  128-slot blocks
    then cut across tile boundaries; a block overlapping k tiles provides k
    matmul "obligations", each with its own one-hot column (other tiles'
    slots set to -1).
  - Each obligation is reduced into its destination tile with a PE matmul
    whose stationary operand is the one-hot (slot -> dest-row) matrix; PSUM
    accumulates all obligations of a tile and the flush applies the
    dest-side norm as a per-partition scale.
  - One-hot matrices are built OH_CH columns at a time with a single DVE
    tensor_tensor(is_equal) against a tiled iota, using a stride-0
    (broadcast) view of the per-column segment values.  Layout
    oh[p, f*OH_CH + j] keeps every operand's last dim contiguous (DVE fast
    modes); the matmul reads column j through a stride-OH_CH view.
  - dma_gather runs in 8-block (1024-idx) chunks — >1024 idxs per call
    wedges the device (2048/4096 verified).  SWDGE descriptor generation
    serializes on the Pool engine at ~2.45us/call, so pass L's chunks are
    emitted first (hidden in the dead window) and a PREFETCH_B window of
    pass-B chunks is emitted next (hidden under AG2).

Host-side work is limited to integer index manipulation (edge partitioning,
sorting, padding, degree counts) and data layout; all fp32 feature math runs
on device.
"""

import sys

if "/opt/trn_rl_repo" not in sys.path:
    sys.path.insert(0, "/opt/trn_rl_repo")

import ml_dtypes
import numpy as np

import concourse.bacc as bacc
import concourse.mybir as mybir
import concourse.tile as tile
from concourse.bass_utils import run_bass_kernel_spmd

# Problem constants (hardcoded per task contract).
N_DRUG = 25000
N_DIS = 25000
N = N_DRUG + N_DIS
E = 800000
D = 128
NCORES = 8
SHARD = N // NCORES           # 6250 rows per core
TILES = (SHARD + 127) // 128  # 49 dest tiles per core
SHARD_PAD = TILES * 128       # 6272
H0T = 25                      # source tiles in AG table 0
H1T = TILES - H0T             # source tiles in AG table 1
H0 = H0T * 128                # 3200 rows
H1 = H1T * 128                # 3072 rows
NPASS = 3                     # L (local), B (table0), C (table1)
CHUNK_BLOCKS = 8              # gather chunk size (blocks of 128 edges);
                              # >1024 idxs in one dma_gather wedges the
                              # device (2048 and 4096 both verified)
PREFETCH_B = 13               # pass-B chunks emitted right after pass L so
                              # their desc-gen hides under the AG2 window
GL_BUFS = 16                  # pass-L pool: holds ALL L chunks (no reuse)
GB_BUFS = 16                  # pass-B pool: prefetch window + streaming
GC_BUFS = 8                   # pass-C pool: streaming only
OH_CH = 8                     # one-hot columns built per DVE instruction
OH_BUFS = 12                  # one-hot pool depth (ring)
OH_PREBUILD = 6               # leading batches emitted before the tile loop
OUT_GROUP = 8                 # dest tiles per output DMA

# Set by test harness to capture a profile; harmless defaults for grading.
TRACE = False
LAST_RESULTS = None

_F32 = mybir.dt.float32
_BF16 = mybir.dt.bfloat16
_I16 = mybir.dt.int16


def _build_nc(geom):
    """Build + compile the SPMD program.

    geom: dict with
      nb[p]       blocks per pass
      obl[t]      per dest tile: list of (pass, block, oh_col, last_use)
                  where last_use marks the final obligation of that block's
                  chunk (frees the chunk buffer)
      n_oh        total one-hot columns
    """
    nb = geom["nb"]
    obl = geom["obl"]
    n_oh = geom["n_oh"]
    nbatch = (n_oh + OH_CH - 1) // OH_CH
    noh_pad = nbatch * OH_CH
    nb_tot = sum(nb)

    nc = bacc.Bacc("TRN2", target_bir_lowering=False, debug=False,
                   num_devices=NCORES, num_swdge_queues=1,
                   dynamic_dma_scratch_size=32768)

    x_t = nc.dram_tensor("xT", [128, SHARD_PAD], _BF16, kind="ExternalInput")
    w = nc.dram_tensor("w", [128, 128], _BF16, kind="ExternalInput")
    deg_d = nc.dram_tensor("deg", [128, TILES], _F32, kind="ExternalInput")
    iota_d = nc.dram_tensor("iota", [128, 128 * OH_CH], _BF16,
                            kind="ExternalInput")
    idxs_d = nc.dram_tensor("idxs", [128, nb_tot * 8], _I16,
                            kind="ExternalInput")
    segs_d = nc.dram_tensor("segs", [128, noh_pad], _BF16,
                            kind="ExternalInput")
    out_d = nc.dram_tensor("out", [SHARD_PAD, 128], _F32, kind="ExternalOutput")

    # pass-L table: this core's own y, written locally after projection.
    y_local = nc.dram_tensor("y_local", [SHARD_PAD, 128], _BF16)
    # AG inputs (partition-major row order) and gathered tables.
    y_half = [nc.dram_tensor("y_half0", [H0, 128], _BF16),
              nc.dram_tensor("y_half1", [H1, 128], _BF16)]
    table = [nc.dram_tensor("table0", [NCORES * H0, 128], _BF16,
                            addr_space="Shared"),
             nc.dram_tensor("table1", [NCORES * H1, 128], _BF16,
                            addr_space="Shared")]

    pass_blk0 = [0, nb[0], nb[0] + nb[1]]
    n_chunks = [(nb[p] + CHUNK_BLOCKS - 1) // CHUNK_BLOCKS
                for p in range(NPASS)]
    chunk_len = [[min(CHUNK_BLOCKS, nb[p] - ci * CHUNK_BLOCKS)
                  for ci in range(n_chunks[p])] for p in range(NPASS)]

    with tile.TileContext(nc) as tc:
        with (
            tc.tile_pool(name="const", bufs=1) as constp,
            tc.tile_pool(name="psum", bufs=8, space="PSUM") as psp,
            tc.tile_pool(name="oh", bufs=OH_BUFS) as ohp,
            tc.tile_pool(name="gl", bufs=GL_BUFS) as glp,
            tc.tile_pool(name="gb", bufs=GB_BUFS) as gbp,
            tc.tile_pool(name="gc", bufs=GC_BUFS) as gcp,
            tc.tile_pool(name="outp", bufs=3) as outp,
        ):
            # ---- projection inputs first (they gate the critical path) ----
            xt = constp.tile([128, SHARD_PAD], _BF16)
            nc.sync.dma_start(xt[:], x_t[:])
            wt = constp.tile([128, 128], _BF16)
            nc.sync.dma_start(wt[:], w[:])
            deg = constp.tile([128, TILES], _F32)
            nc.sync.dma_start(deg[:], deg_d[:])

            # ---- norm = rsqrt(max(deg, 1)) ----
            dmax = constp.tile([128, TILES], _F32)
            nc.vector.tensor_scalar_max(dmax[:], deg[:], 1.0)
            rcp = constp.tile([128, TILES], _F32)
            nc.vector.reciprocal(rcp[:], dmax[:])
            norm = constp.tile([128, TILES], _F32)
            nc.scalar.sqrt(norm[:], rcp[:])

            # ---- gather-phase index inputs (small; loaded early so DVE
            # one-hot prebuild + pass-L desc-gen start in the dead window) --
            iota = constp.tile([128, 128 * OH_CH], _BF16)
            nc.sync.dma_start(iota[:], iota_d[:])
            segs = constp.tile([128, noh_pad], _BF16)
            nc.sync.dma_start(segs[:], segs_d[:])
            idxs = constp.tile([128, nb_tot * 8], _I16)
            nc.sync.dma_start(idxs[:], idxs_d[:])

            # ---- projection: ybuf[:, t, :] = bf16((X @ W) * norm) ----
            ybuf = constp.tile([128, TILES, 128], _BF16)
            for t in range(TILES):
                ps = psp.tile([128, 128], _F32)
                nc.tensor.matmul(ps[:], xt[:, t * 128:(t + 1) * 128], wt[:],
                                 start=True, stop=True)
                nc.scalar.activation(ybuf[:, t, :], ps[:],
                                     mybir.ActivationFunctionType.Copy,
                                     scale=norm[:, t:t + 1])
                if t == H0T - 1:
                    nc.sync.dma_start(
                        y_half[0].ap().rearrange("(p t) o -> p t o", p=128),
                        ybuf[:, 0:H0T, :])
                    nc.gpsimd.collective_compute(
                        "AllGather", mybir.AluOpType.bypass,
                        replica_groups=[list(range(NCORES))],
                        ins=[y_half[0].ap()], outs=[table[0].ap()])
            # local table write (pass L reads it; no collective involved)
            nc.sync.dma_start(
                y_local.ap().rearrange("(p t) o -> p t o", p=128),
                ybuf[:, :, :])
            nc.sync.dma_start(
                y_half[1].ap().rearrange("(p t) o -> p t o", p=128),
                ybuf[:, H0T:, :])
            nc.gpsimd.collective_compute(
                "AllGather", mybir.AluOpType.bypass,
                replica_groups=[list(range(NCORES))],
                ins=[y_half[1].ap()], outs=[table[1].ap()])

            # ---- one-hot batches: OH_CH columns per DVE op ----
            ohbufs = {}

            def ensure_ohbatch(bi):
                if bi in ohbufs:
                    return
                oh = ohp.tile([128, 128 * OH_CH], _BF16, tag="oh")
                o_v = oh[:].rearrange("p (f j) -> p f j", j=OH_CH)
                i_v = iota[:].rearrange("p (f j) -> p f j", j=OH_CH)
                s_v = (segs[:, bi * OH_CH:(bi + 1) * OH_CH]
                       .unsqueeze(1).broadcast_to([128, 128, OH_CH]))
                nc.vector.tensor_tensor(o_v, i_v, s_v,
                                        mybir.AluOpType.is_equal)
                ohbufs[bi] = oh

            for k in range(OH_PREBUILD):
                ensure_ohbatch(k)

            # ---- gathers: pass L first (no AG dep -> runs in the dead
            # window), then a pass-B prefetch window (hides under AG2),
            # then on-demand in consumption order ----
            gbufs = [[None] * n_chunks[p] for p in range(NPASS)]
            pools = [glp, gbp, gcp]
            srcs = [y_local[:, :], table[0][:, :], table[1][:, :]]
            emit_counter = [0]

            def ensure_chunk(p, ci):
                if gbufs[p][ci] is not None:
                    return
                cl = chunk_len[p][ci]
                gb = pools[p].tile([128, CHUNK_BLOCKS, 128], _BF16,
                                   tag="g%d" % p)
                col0 = (pass_blk0[p] + ci * CHUNK_BLOCKS) * 8
                nc.gpsimd.dma_gather(
                    gb[:, 0:cl, :],
                    srcs[p],
                    idxs[:, col0:col0 + cl * 8],
                    cl * 128,
                    cl * 128,
                    128,
                    queue_num=0,
                )
                emit_counter[0] += 1
                gbufs[p][ci] = gb

            assert n_chunks[0] <= GL_BUFS, (n_chunks[0], GL_BUFS)
            for ci in range(n_chunks[0]):
                ensure_chunk(0, ci)
            for ci in range(min(PREFETCH_B, n_chunks[1])):
                ensure_chunk(1, ci)

            out_v = out_d.ap().rearrange("(p t) o -> p t o", p=128)
            for t in range(TILES):
                obligations = obl[t]
                ps = psp.tile([128, 128], _F32)
                for k, (p, blk, g, last_use) in enumerate(obligations):
                    ci, slot = divmod(blk, CHUNK_BLOCKS)
                    ensure_chunk(p, ci)
                    bi, jj = divmod(g, OH_CH)
                    ensure_ohbatch(bi)
                    lhsT = (ohbufs[bi][:]
                            .rearrange("p (f j) -> p f j", j=OH_CH)[:, :, jj])
                    nc.tensor.matmul(ps[:], lhsT, gbufs[p][ci][:, slot, :],
                                     start=(k == 0),
                                     stop=(k == len(obligations) - 1))
                    if last_use:
                        gbufs[p][ci] = None
                g0 = (t // OUT_GROUP) * OUT_GROUP
                if t == g0:
                    obuf = outp.tile([128, min(OUT_GROUP, TILES - g0), 128],
                                     _F32)
                nc.scalar.activation(obuf[:, t - g0, :], ps[:],
                                     mybir.ActivationFunctionType.Copy,
                                     scale=norm[:, t:t + 1])
                if t == min(g0 + OUT_GROUP, TILES) - 1:
                    nc.sync.dma_start(out_v[:, g0:t + 1, :], obuf[:])

    nc.compile()
    return nc


def _preprocess(rows, cols):
    """Partition edges into 3 passes with packed slot layout.

    Returns (geom, deg, per_core) where per_core[c] = (idx_tile, seg_tile).
    """
    rows = np.asarray(rows, dtype=np.int64)
    cols = np.asarray(cols, dtype=np.int64)

    deg = np.bincount(rows, minlength=N).astype(np.float32)

    core = rows // SHARD
    t_of = (rows - core * SHARD) >> 7
    seg_of = (rows - core * SHARD) & 127
    src_core = cols // SHARD
    src_l = cols - src_core * SHARD
    src_t = src_l >> 7
    src_p = src_l & 127

    # table index per edge for each possible pass:
    #   L: local y table, partition-major: idx = src_p*TILES + src_t
    #   B: idx = src_core*H0 + src_p*H0T + src_t
    #   C: idx = src_core*H1 + src_p*H1T + (src_t - H0T)
    local = src_core == core
    p_of = np.where(local, 0, np.where(src_t < H0T, 1, 2))
    idx_of = np.where(
        local, src_p * TILES + src_t,
        np.where(src_t < H0T,
                 src_core * H0 + src_p * H0T + src_t,
                 src_core * H1 + src_p * H1T + (src_t - H0T)))

    # per (core, pass, tile) counts -> shared packed layout M/cum
    key = (core * NPASS + p_of) * TILES + t_of
    counts = np.bincount(key, minlength=NCORES * NPASS * TILES).reshape(
        NCORES, NPASS, TILES)
    M = counts.max(axis=0)                      # [NPASS][TILES]
    M[0] = np.maximum(M[0], 1)  # every tile has >=1 slot so PSUM is defined

    nb = []
    cum = []
    for p in range(NPASS):
        c = np.zeros(TILES + 1, np.int64)
        c[1:] = np.cumsum(M[p])
        cum.append(c)
        nb.append(int((c[TILES] + 127) // 128))
    nb_tot = sum(nb)
    pass_blk0 = [0, nb[0], nb[0] + nb[1]]
    pass_slot0 = [0, nb[0] * 128, (nb[0] + nb[1]) * 128]

    # obligations: for each pass/tile, blocks overlapped by its slot range.
    # oh column g numbering: in (tile, pass, block) consumption order.
    obl = [[] for _ in range(TILES)]            # t -> [(p, blk, g, last)]
    n_oh = 0
    chunk_last = {}  # (p, chunk) -> flat position of final obligation
    flat_pos = 0
    obl_meta = []    # (p, t, blk, g, lo, hi) for host seg fill
    for t in range(TILES):
        for p in range(NPASS):
            lo, hi = int(cum[p][t]), int(cum[p][t + 1])
            if hi == lo:
                continue
            b0, b1 = lo // 128, (hi - 1) // 128
            for blk in range(b0, b1 + 1):
                s_lo, s_hi = max(lo, blk * 128), min(hi, (blk + 1) * 128)
                g = n_oh
                n_oh += 1
                obl[t].append([p, blk, g, False])
                obl_meta.append((p, t, blk, g, s_lo, s_hi))
                chunk_last[(p, blk // CHUNK_BLOCKS)] = flat_pos
                flat_pos += 1
    # mark last use of each chunk
    pos = 0
    for t in range(TILES):
        for o in obl[t]:
            if chunk_last[(o[0], o[1] // CHUNK_BLOCKS)] == pos:
                o[3] = True
            pos += 1
    obl = [[tuple(o) for o in obl_t] for obl_t in obl]

    nbatch = (n_oh + OH_CH - 1) // OH_CH
    noh_pad = nbatch * OH_CH

    # shared seg template: seg values depend on slot fills, which differ per
    # core only in WHICH slots are padded (-1); the one-hot column for
    # (p, blk, t) masks other tiles' slots to -1 for every core identically
    # only where the core has no edge.  So segs must be built per core.
    order = np.argsort(key, kind="stable")

    per_core = []
    for c in range(NCORES):
        slot_seg = np.full(nb_tot * 128, -1.0, np.float32)
        slot_idx = np.zeros(nb_tot * 128, np.int16)
        sel = order[(core[order] == c).nonzero()[0]]
        kk = key[sel] - c * NPASS * TILES       # p*TILES + t, sorted
        p_sel = kk // TILES
        t_sel = kk % TILES
        grp_start = np.searchsorted(kk, np.arange(NPASS * TILES), side="left")
        pos_in_grp = np.arange(sel.size) - grp_start[kk]
        base = (np.take(pass_slot0, p_sel)
                + cum[0][t_sel] * (p_sel == 0)
                + cum[1][t_sel] * (p_sel == 1)
                + cum[2][t_sel] * (p_sel == 2))
        dst = base + pos_in_grp
        slot_idx[dst] = idx_of[sel].astype(np.int16)
        slot_seg[dst] = seg_of[sel].astype(np.float32)

        # seg columns per obligation: slice [s_lo, s_hi) of the block,
        # other slots -1.
        seg_cols = np.full((noh_pad, 128), -1.0, np.float32)
        for (p, t, blk, g, s_lo, s_hi) in obl_meta:
            sb = pass_slot0[p] + blk * 128
            a = s_lo - blk * 128
            b = s_hi - blk * 128
            seg_cols[g, a:b] = slot_seg[sb + a: sb + b]

        idx_tile = np.tile(slot_idx.reshape(nb_tot * 8, 16).T, (8, 1))
        seg_tile = np.ascontiguousarray(seg_cols.T).astype(ml_dtypes.bfloat16)
        per_core.append((idx_tile, seg_tile))

    geom = {"nb": nb, "obl": obl, "n_oh": n_oh}
    return geom, deg, per_core


def kernel(drug_f, disease_f, drug_w, disease_w, rows, cols):
    global LAST_RESULTS
    drug_f = np.asarray(drug_f, np.float32)
    disease_f = np.asarray(disease_f, np.float32)
    drug_w = np.asarray(drug_w, np.float32)
    disease_w = np.asarray(disease_w, np.float32)

    geom, deg, per_core = _preprocess(rows, cols)
    nc = _build_nc(geom)

    feats = np.concatenate([drug_f, disease_f], axis=0)  # [N, 128]
    # iota[p, f*OH_CH + j] = f
    iota = np.repeat(np.arange(128, dtype=np.float32), OH_CH)
    iota = np.tile(iota[None, :], (128, 1)).astype(ml_dtypes.bfloat16)

    in_maps = []
    for c in range(NCORES):
        sh = feats[c * SHARD:(c + 1) * SHARD]           # [6250, 128]
        x_t = np.zeros((128, SHARD_PAD), ml_dtypes.bfloat16)
        x_t[:, :SHARD] = sh.T.astype(ml_dtypes.bfloat16)
        dg = np.ones((SHARD_PAD,), np.float32)
        dg[:SHARD] = deg[c * SHARD:(c + 1) * SHARD]
        idx_tile, seg_tile = per_core[c]
        in_maps.append({
            "xT": x_t,
            "w": (drug_w if c < 4 else disease_w).astype(ml_dtypes.bfloat16),
            "deg": dg.reshape(TILES, 128).T.copy(),
            "iota": iota,
            "idxs": idx_tile,
            "segs": seg_tile,
        })

    res = run_bass_kernel_spmd(nc, in_maps, core_ids=list(range(NCORES)),
                               trace=TRACE)
    LAST_RESULTS = res

    # out rows are partition-major (p*TILES + t); restore node order.
    outs = []
    for c in range(NCORES):
        r = res.results[c]["out"].reshape(128, TILES, 128)
        outs.append(r.transpose(1, 0, 2).reshape(SHARD_PAD, 128)[:SHARD])
    return np.concatenate(outs, axis=0)


# revision 30
# speedup vs baseline: 2.3685x; 2.3685x over previous
"""Trainium2 Bass kernel for a 2-relation DGL-style GNN layer.

Math (see reference):
    h   = concat(drug_f @ drug_w, disease_f @ disease_w)        [N, 128]
    deg = bincount(rows); norm = clip(deg,1)^-0.5
    out = norm * segsum(  (norm * h)[cols], rows )

Distribution (8 NeuronCores, SPMD):
  - Nodes row-sharded: core c owns rows [c*6250, (c+1)*6250).  Cores 0-3 hold
    drug rows, cores 4-7 disease rows, so each core projects its shard with a
    single 128x128 weight (supplied per-core), scales it by the source norm,
    and quantizes to bf16 (y).  y rows are PARTITION-MAJOR (row = p*tiles + t
    for node t*128+p) so each y write is one contiguous descriptor per
    partition.
  - Three gather passes, partitioned by edge SOURCE:
      pass 0 (L): source in this core's own shard -> gathered from y_local,
        written right after projection (~22us).  No collective dependency,
        so its SWDGE descriptor generation and DMA transfers run during the
        ~66us CC-engine init + AllGather dead window.
      pass 1 (B): remote sources with src tile in [0,25) -> table0 (AG1).
      pass 2 (C): remote sources with src tile in [25,49) -> table1 (AG2).
    Tables are < 2^15 rows so int16 gather indices address them directly.
  - Edge slots are PACKED: per pass, tile t owns slot range
    [cum[t], cum[t]+M[t]) where M[t] = max over cores of the core's edge
    count for (pass, t) — no rounding of each tile to 128.  128-slot blocks
    then cut across tile boundaries; a block overlapping k tiles provides k
    matmul "obligations", each with its own one-hot column (other tiles'
    slots set to -1).
  - Each obligation is reduced into its destination tile with a PE matmul
    whose stationary operand is the one-hot (slot -> dest-row) matrix; PSUM
    accumulates all obligations of a tile and the flush applies the
    dest-side norm as a per-partition scale.
  - One-hot matrices are built OH_CH columns at a time with a single DVE
    tensor_tensor(is_equal) against a tiled iota, using a stride-0
    (broadcast) view of the per-column segment values.  Layout
    oh[p, f*OH_CH + j] keeps every operand's last dim contiguous (DVE fast
    modes); the matmul reads column j through a stride-OH_CH view.
  - dma_gather runs in 8-block (1024-idx) chunks — >1024 idxs per call
    wedges the device (2048/4096 verified).  SWDGE descriptor generation
    serializes on the Pool engine at ~2.45us/call, so pass L's chunks are
    emitted first (hidden in the dead window) and a PREFETCH_B window of
    pass-B chunks is emitted next (hidden under AG2).

Host-side work is limited to integer index manipulation (edge partitioning,
sorting, padding, degree counts) and data layout; all fp32 feature math runs
on device.
"""

import sys

if "/opt/trn_rl_repo" not in sys.path:
    sys.path.insert(0, "/opt/trn_rl_repo")

import ml_dtypes
import numpy as np

import concourse.bacc as bacc
import concourse.mybir as mybir
import concourse.tile as tile
from concourse.bass_utils import run_bass_kernel_spmd

# Problem constants (hardcoded per task contract).
N_DRUG = 25000
N_DIS = 25000
N = N_DRUG + N_DIS
E = 800000
D = 128
NCORES = 8
SHARD = N // NCORES           # 6250 rows per core
TILES = (SHARD + 127) // 128  # 49 dest tiles per core
SHARD_PAD = TILES * 128       # 6272
H0T = 20                      # source tiles in AG table 0
H1T = TILES - H0T             # source tiles in AG table 1
H0 = H0T * 128                # 3200 rows
H1 = H1T * 128                # 3072 rows
NPASS = 3                     # L (local), B (table0), C (table1)
CHUNK_BLOCKS = 8              # gather chunk size (blocks of 128 edges);
                              # >1024 idxs in one dma_gather wedges the
                              # device (2048 and 4096 both verified)
PREFETCH_B = 16               # pass-B chunks emitted right after pass L so
                              # their desc-gen hides under the AG2 window
GL_BUFS = 16                  # pass-L pool: holds ALL L chunks (no reuse)
GB_BUFS = 16                  # pass-B pool: prefetch window + streaming
GC_BUFS = 8                   # pass-C pool: streaming only
OH_CH = 8                     # one-hot columns built per DVE instruction
OH_BUFS = 12                  # one-hot pool depth (ring)
OH_PREBUILD = 6               # leading batches emitted before the tile loop
OUT_GROUP = 8                 # dest tiles per output DMA

# Set by test harness to capture a profile; harmless defaults for grading.
TRACE = False
LAST_RESULTS = None

_F32 = mybir.dt.float32
_BF16 = mybir.dt.bfloat16
_I16 = mybir.dt.int16


def _build_nc(geom):
    """Build + compile the SPMD program.

    geom: dict with
      nb[p]       blocks per pass
      obl[t]      per dest tile: list of (pass, block, oh_col, last_use)
                  where last_use marks the final obligation of that block's
                  chunk (frees the chunk buffer)
      n_oh        total one-hot columns
    """
    nb = geom["nb"]
    obl = geom["obl"]
    n_oh = geom["n_oh"]
    nbatch = (n_oh + OH_CH - 1) // OH_CH
    noh_pad = nbatch * OH_CH
    nb_tot = sum(nb)

    nc = bacc.Bacc("TRN2", target_bir_lowering=False, debug=False,
                   num_devices=NCORES, num_swdge_queues=4,
                   dynamic_dma_scratch_size=32768)

    x_t = nc.dram_tensor("xT", [128, SHARD_PAD], _BF16, kind="ExternalInput")
    w = nc.dram_tensor("w", [128, 128], _BF16, kind="ExternalInput")
    deg_d = nc.dram_tensor("deg", [128, TILES], _F32, kind="ExternalInput")
    iota_d = nc.dram_tensor("iota", [128, 128 * OH_CH], _BF16,
                            kind="ExternalInput")
    idxs_d = nc.dram_tensor("idxs", [128, nb_tot * 8], _I16,
                            kind="ExternalInput")
    segs_d = nc.dram_tensor("segs", [128, noh_pad], _BF16,
                            kind="ExternalInput")
    out_d = nc.dram_tensor("out", [SHARD_PAD, 128], _F32, kind="ExternalOutput")

    # pass-L table: this core's own y, written locally after projection.
    y_local = nc.dram_tensor("y_local", [SHARD_PAD, 128], _BF16)
    # AG inputs (partition-major row order) and gathered tables.
    y_half = [nc.dram_tensor("y_half0", [H0, 128], _BF16),
              nc.dram_tensor("y_half1", [H1, 128], _BF16)]
    table = [nc.dram_tensor("table0", [NCORES * H0, 128], _BF16,
                            addr_space="Shared"),
             nc.dram_tensor("table1", [NCORES * H1, 128], _BF16,
                            addr_space="Shared")]

    pass_blk0 = [0, nb[0], nb[0] + nb[1]]
    n_chunks = [(nb[p] + CHUNK_BLOCKS - 1) // CHUNK_BLOCKS
                for p in range(NPASS)]
    chunk_len = [[min(CHUNK_BLOCKS, nb[p] - ci * CHUNK_BLOCKS)
                  for ci in range(n_chunks[p])] for p in range(NPASS)]

    with tile.TileContext(nc) as tc:
        with (
            tc.tile_pool(name="const", bufs=1) as constp,
            tc.tile_pool(name="psum", bufs=8, space="PSUM") as psp,
            tc.tile_pool(name="oh", bufs=OH_BUFS) as ohp,
            tc.tile_pool(name="gl", bufs=GL_BUFS) as glp,
            tc.tile_pool(name="gb", bufs=GB_BUFS) as gbp,
            tc.tile_pool(name="gc", bufs=GC_BUFS) as gcp,
            tc.tile_pool(name="outp", bufs=3) as outp,
        ):
            # ---- projection inputs first (they gate the critical path) ----
            xt = constp.tile([128, SHARD_PAD], _BF16)
            nc.sync.dma_start(xt[:], x_t[:])
            wt = constp.tile([128, 128], _BF16)
            nc.sync.dma_start(wt[:], w[:])
            deg = constp.tile([128, TILES], _F32)
            nc.sync.dma_start(deg[:], deg_d[:])

            # ---- norm = rsqrt(max(deg, 1)) ----
            dmax = constp.tile([128, TILES], _F32)
            nc.vector.tensor_scalar_max(dmax[:], deg[:], 1.0)
            rcp = constp.tile([128, TILES], _F32)
            nc.vector.reciprocal(rcp[:], dmax[:])
            norm = constp.tile([128, TILES], _F32)
            nc.scalar.sqrt(norm[:], rcp[:])

            # ---- gather-phase index inputs (small; loaded early so DVE
            # one-hot prebuild + pass-L desc-gen start in the dead window) --
            iota = constp.tile([128, 128 * OH_CH], _BF16)
            nc.sync.dma_start(iota[:], iota_d[:])
            segs = constp.tile([128, noh_pad], _BF16)
            nc.sync.dma_start(segs[:], segs_d[:])
            idxs = constp.tile([128, nb_tot * 8], _I16)
            nc.sync.dma_start(idxs[:], idxs_d[:])

            # ---- projection: ybuf[:, t, :] = bf16((X @ W) * norm) ----
            ybuf = constp.tile([128, TILES, 128], _BF16)
            for t in range(TILES):
                ps = psp.tile([128, 128], _F32)
                nc.tensor.matmul(ps[:], xt[:, t * 128:(t + 1) * 128], wt[:],
                                 start=True, stop=True)
                nc.scalar.activation(ybuf[:, t, :], ps[:],
                                     mybir.ActivationFunctionType.Copy,
                                     scale=norm[:, t:t + 1])
                if t == H0T - 1:
                    nc.sync.dma_start(
                        y_half[0].ap().rearrange("(p t) o -> p t o", p=128),
                        ybuf[:, 0:H0T, :])
                    nc.gpsimd.collective_compute(
                        "AllGather", mybir.AluOpType.bypass,
                        replica_groups=[list(range(NCORES))],
                        ins=[y_half[0].ap()], outs=[table[0].ap()])
            # local table write (pass L reads it; no collective involved)
            nc.sync.dma_start(
                y_local.ap().rearrange("(p t) o -> p t o", p=128),
                ybuf[:, :, :])
            nc.sync.dma_start(
                y_half[1].ap().rearrange("(p t) o -> p t o", p=128),
                ybuf[:, H0T:, :])
            nc.gpsimd.collective_compute(
                "AllGather", mybir.AluOpType.bypass,
                replica_groups=[list(range(NCORES))],
                ins=[y_half[1].ap()], outs=[table[1].ap()])

            # ---- one-hot batches: OH_CH columns per DVE op ----
            ohbufs = {}

            def ensure_ohbatch(bi):
                if bi in ohbufs:
                    return
                oh = ohp.tile([128, 128 * OH_CH], _BF16, tag="oh")
                o_v = oh[:].rearrange("p (f j) -> p f j", j=OH_CH)
                i_v = iota[:].rearrange("p (f j) -> p f j", j=OH_CH)
                s_v = (segs[:, bi * OH_CH:(bi + 1) * OH_CH]
                       .unsqueeze(1).broadcast_to([128, 128, OH_CH]))
                nc.vector.tensor_tensor(o_v, i_v, s_v,
                                        mybir.AluOpType.is_equal)
                ohbufs[bi] = oh

            for k in range(OH_PREBUILD):
                ensure_ohbatch(k)

            # ---- gathers: pass L first (no AG dep -> runs in the dead
            # window), then a pass-B prefetch window (hides under AG2),
            # then on-demand in consumption order ----
            gbufs = [[None] * n_chunks[p] for p in range(NPASS)]
            pools = [glp, gbp, gcp]
            srcs = [y_local[:, :], table[0][:, :], table[1][:, :]]
            emit_counter = [0]

            def ensure_chunk(p, ci):
                if gbufs[p][ci] is not None:
                    return
                cl = chunk_len[p][ci]
                gb = pools[p].tile([128, CHUNK_BLOCKS, 128], _BF16,
                                   tag="g%d" % p)
                col0 = (pass_blk0[p] + ci * CHUNK_BLOCKS) * 8
                nc.gpsimd.dma_gather(
                    gb[:, 0:cl, :],
                    srcs[p],
                    idxs[:, col0:col0 + cl * 8],
                    cl * 128,
                    cl * 128,
                    128,
                    queue_num=emit_counter[0] % 4,
                )
                emit_counter[0] += 1
                gbufs[p][ci] = gb

            assert n_chunks[0] <= GL_BUFS, (n_chunks[0], GL_BUFS)
            for ci in range(n_chunks[0]):
                ensure_chunk(0, ci)
            for ci in range(min(PREFETCH_B, n_chunks[1])):
                ensure_chunk(1, ci)

            out_v = out_d.ap().rearrange("(p t) o -> p t o", p=128)
            for t in range(TILES):
                obligations = obl[t]
                ps = psp.tile([128, 128], _F32)
                for k, (p, blk, g, last_use) in enumerate(obligations):
                    ci, slot = divmod(blk, CHUNK_BLOCKS)
                    ensure_chunk(p, ci)
                    bi, jj = divmod(g, OH_CH)
                    ensure_ohbatch(bi)
                    lhsT = (ohbufs[bi][:]
                            .rearrange("p (f j) -> p f j", j=OH_CH)[:, :, jj])
                    nc.tensor.matmul(ps[:], lhsT, gbufs[p][ci][:, slot, :],
                                     start=(k == 0),
                                     stop=(k == len(obligations) - 1))
                    if last_use:
                        gbufs[p][ci] = None
                g0 = (t // OUT_GROUP) * OUT_GROUP
                if t == g0:
                    obuf = outp.tile([128, min(OUT_GROUP, TILES - g0), 128],
                                     _F32)
                nc.scalar.activation(obuf[:, t - g0, :], ps[:],
                                     mybir.ActivationFunctionType.Copy,
                                     scale=norm[:, t:t + 1])
                if t == min(g0 + OUT_GROUP, TILES) - 1:
                    nc.sync.dma_start(out_v[:, g0:t + 1, :], obuf[:])

    nc.compile()
    return nc


def _preprocess(rows, cols):
    """Partition edges into 3 passes with packed slot layout.

    Returns (geom, deg, per_core) where per_core[c] = (idx_tile, seg_tile).
    """
    rows = np.asarray(rows, dtype=np.int64)
    cols = np.asarray(cols, dtype=np.int64)

    deg = np.bincount(rows, minlength=N).astype(np.float32)

    core = rows // SHARD
    t_of = (rows - core * SHARD) >> 7
    seg_of = (rows - core * SHARD) & 127
    src_core = cols // SHARD
    src_l = cols - src_core * SHARD
    src_t = src_l >> 7
    src_p = src_l & 127

    # table index per edge for each possible pass:
    #   L: local y table, partition-major: idx = src_p*TILES + src_t
    #   B: idx = src_core*H0 + src_p*H0T + src_t
    #   C: idx = src_core*H1 + src_p*H1T + (src_t - H0T)
    local = src_core == core
    p_of = np.where(local, 0, np.where(src_t < H0T, 1, 2))
    idx_of = np.where(
        local, src_p * TILES + src_t,
        np.where(src_t < H0T,
                 src_core * H0 + src_p * H0T + src_t,
                 src_core * H1 + src_p * H1T + (src_t - H0T)))

    # per (core, pass, tile) counts -> shared packed layout M/cum
    key = (core * NPASS + p_of) * TILES + t_of
    counts = np.bincount(key, minlength=NCORES * NPASS * TILES).reshape(
        NCORES, NPASS, TILES)
    M = counts.max(axis=0)                      # [NPASS][TILES]
    M[0] = np.maximum(M[0], 1)  # every tile has >=1 slot so PSUM is defined

    nb = []
    cum = []
    for p in range(NPASS):
        c = np.zeros(TILES + 1, np.int64)
        c[1:] = np.cumsum(M[p])
        cum.append(c)
        nb.append(int((c[TILES] + 127) // 128))
    nb_tot = sum(nb)
    pass_blk0 = [0, nb[0], nb[0] + nb[1]]
    pass_slot0 = [0, nb[0] * 128, (nb[0] + nb[1]) * 128]

    # obligations: for each pass/tile, blocks overlapped by its slot range.
    # oh column g numbering: in (tile, pass, block) consumption order.
    obl = [[] for _ in range(TILES)]            # t -> [(p, blk, g, last)]
    n_oh = 0
    chunk_last = {}  # (p, chunk) -> flat position of final obligation
    flat_pos = 0
    obl_meta = []    # (p, t, blk, g, lo, hi) for host seg fill
    for t in range(TILES):
        for p in range(NPASS):
            lo, hi = int(cum[p][t]), int(cum[p][t + 1])
            if hi == lo:
                continue
            b0, b1 = lo // 128, (hi - 1) // 128
            for blk in range(b0, b1 + 1):
                s_lo, s_hi = max(lo, blk * 128), min(hi, (blk + 1) * 128)
                g = n_oh
                n_oh += 1
                obl[t].append([p, blk, g, False])
                obl_meta.append((p, t, blk, g, s_lo, s_hi))
                chunk_last[(p, blk // CHUNK_BLOCKS)] = flat_pos
                flat_pos += 1
    # mark last use of each chunk
    pos = 0
    for t in range(TILES):
        for o in obl[t]:
            if chunk_last[(o[0], o[1] // CHUNK_BLOCKS)] == pos:
                o[3] = True
            pos += 1
    obl = [[tuple(o) for o in obl_t] for obl_t in obl]

    nbatch = (n_oh + OH_CH - 1) // OH_CH
    noh_pad = nbatch * OH_CH

    # shared seg template: seg values depend on slot fills, which differ per
    # core only in WHICH slots are padded (-1); the one-hot column for
    # (p, blk, t) masks other tiles' slots to -1 for every core identically
    # only where the core has no edge.  So segs must be built per core.
    order = np.argsort(key, kind="stable")

    per_core = []
    for c in range(NCORES):
        slot_seg = np.full(nb_tot * 128, -1.0, np.float32)
        slot_idx = np.zeros(nb_tot * 128, np.int16)
        sel = order[(core[order] == c).nonzero()[0]]
        kk = key[sel] - c * NPASS * TILES       # p*TILES + t, sorted
        p_sel = kk // TILES
        t_sel = kk % TILES
        grp_start = np.searchsorted(kk, np.arange(NPASS * TILES), side="left")
        pos_in_grp = np.arange(sel.size) - grp_start[kk]
        base = (np.take(pass_slot0, p_sel)
                + cum[0][t_sel] * (p_sel == 0)
                + cum[1][t_sel] * (p_sel == 1)
                + cum[2][t_sel] * (p_sel == 2))
        dst = base + pos_in_grp
        slot_idx[dst] = idx_of[sel].astype(np.int16)
        slot_seg[dst] = seg_of[sel].astype(np.float32)

        # seg columns per obligation: slice [s_lo, s_hi) of the block,
        # other slots -1.
        seg_cols = np.full((noh_pad, 128), -1.0, np.float32)
        for (p, t, blk, g, s_lo, s_hi) in obl_meta:
            sb = pass_slot0[p] + blk * 128
            a = s_lo - blk * 128
            b = s_hi - blk * 128
            seg_cols[g, a:b] = slot_seg[sb + a: sb + b]

        idx_tile = np.tile(slot_idx.reshape(nb_tot * 8, 16).T, (8, 1))
        seg_tile = np.ascontiguousarray(seg_cols.T).astype(ml_dtypes.bfloat16)
        per_core.append((idx_tile, seg_tile))

    geom = {"nb": nb, "obl": obl, "n_oh": n_oh}
    return geom, deg, per_core


def kernel(drug_f, disease_f, drug_w, disease_w, rows, cols):
    global LAST_RESULTS
    drug_f = np.asarray(drug_f, np.float32)
    disease_f = np.asarray(disease_f, np.float32)
    drug_w = np.asarray(drug_w, np.float32)
    disease_w = np.asarray(disease_w, np.float32)

    geom, deg, per_core = _preprocess(rows, cols)
    nc = _build_nc(geom)

    feats = np.concatenate([drug_f, disease_f], axis=0)  # [N, 128]
    # iota[p, f*OH_CH + j] = f
    iota = np.repeat(np.arange(128, dtype=np.float32), OH_CH)
    iota = np.tile(iota[None, :], (128, 1)).astype(ml_dtypes.bfloat16)

    in_maps = []
    for c in range(NCORES):
        sh = feats[c * SHARD:(c + 1) * SHARD]           # [6250, 128]
        x_t = np.zeros((128, SHARD_PAD), ml_dtypes.bfloat16)
        x_t[:, :SHARD] = sh.T.astype(ml_dtypes.bfloat16)
        dg = np.ones((SHARD_PAD,), np.float32)
        dg[:SHARD] = deg[c * SHARD:(c + 1) * SHARD]
        idx_tile, seg_tile = per_core[c]
        in_maps.append({
            "xT": x_t,
            "w": (drug_w if c < 4 else disease_w).astype(ml_dtypes.bfloat16),
            "deg": dg.reshape(TILES, 128).T.copy(),
            "iota": iota,
            "idxs": idx_tile,
            "segs": seg_tile,
        })

    res = run_bass_kernel_spmd(nc, in_maps, core_ids=list(range(NCORES)),
                               trace=TRACE)
    LAST_RESULTS = res

    # out rows are partition-major (p*TILES + t); restore node order.
    outs = []
    for c in range(NCORES):
        r = res.results[c]["out"].reshape(128, TILES, 128)
        outs.append(r.transpose(1, 0, 2).reshape(SHARD_PAD, 128)[:SHARD])
    return np.concatenate(outs, axis=0)


# revision 31
# speedup vs baseline: 2.4715x; 1.0435x over previous
"""Trainium2 Bass kernel for a 2-relation DGL-style GNN layer.

Math (see reference):
    h   = concat(drug_f @ drug_w, disease_f @ disease_w)        [N, 128]
    deg = bincount(rows); norm = clip(deg,1)^-0.5
    out = norm * segsum(  (norm * h)[cols], rows )

Distribution (8 NeuronCores, SPMD):
  - Nodes row-sharded: core c owns rows [c*6250, (c+1)*6250).  Cores 0-3 hold
    drug rows, cores 4-7 disease rows, so each core projects its shard with a
    single 128x128 weight (supplied per-core), scales it by the source norm,
    and quantizes to bf16 (y).  y rows are PARTITION-MAJOR (row = p*tiles + t
    for node t*128+p) so each y write is one contiguous descriptor per
    partition.
  - Three gather passes, partitioned by edge SOURCE:
      pass 0 (L): source in this core's own shard -> gathered from y_local,
        written right after projection (~22us).  No collective dependency,
        so its SWDGE descriptor generation and DMA transfers run during the
        ~66us CC-engine init + AllGather dead window.
      pass 1 (B): remote sources with src tile in [0,25) -> table0 (AG1).
      pass 2 (C): remote sources with src tile in [25,49) -> table1 (AG2).
    Tables are < 2^15 rows so int16 gather indices address them directly.
  - Edge slots are PACKED: per pass, tile t owns slot range
    [cum[t], cum[t]+M[t]) where M[t] = max over cores of the core's edge
    count for (pass, t) — no rounding of each tile to 128.  128-slot blocks
    then cut across tile boundaries; a block overlapping k tiles provides k
    matmul "obligations", each with its own one-hot column (other tiles'
    slots set to -1).
  - Each obligation is reduced into its destination tile with a PE matmul
    whose stationary operand is the one-hot (slot -> dest-row) matrix; PSUM
    accumulates all obligations of a tile and the flush applies the
    dest-side norm as a per-partition scale.
  - One-hot matrices are built OH_CH columns at a time with a single DVE
    tensor_tensor(is_equal) against a tiled iota, using a stride-0
    (broadcast) view of the per-column segment values.  Layout
    oh[p, f*OH_CH + j] keeps every operand's last dim contiguous (DVE fast
    modes); the matmul reads column j through a stride-OH_CH view.
  - dma_gather runs in 8-block (1024-idx) chunks — >1024 idxs per call
    wedges the device (2048/4096 verified).  SWDGE descriptor generation
    serializes on the Pool engine at ~2.45us/call, so pass L's chunks are
    emitted first (hidden in the dead window) and a PREFETCH_B window of
    pass-B chunks is emitted next (hidden under AG2).

Host-side work is limited to integer index manipulation (edge partitioning,
sorting, padding, degree counts) and data layout; all fp32 feature math runs
on device.
"""

import sys

if "/opt/trn_rl_repo" not in sys.path:
    sys.path.insert(0, "/opt/trn_rl_repo")

import ml_dtypes
import numpy as np

import concourse.bacc as bacc
import concourse.mybir as mybir
import concourse.tile as tile
from concourse.bass_utils import run_bass_kernel_spmd

# Problem constants (hardcoded per task contract).
N_DRUG = 25000
N_DIS = 25000
N = N_DRUG + N_DIS
E = 800000
D = 128
NCORES = 8
SHARD = N // NCORES           # 6250 rows per core
TILES = (SHARD + 127) // 128  # 49 dest tiles per core
SHARD_PAD = TILES * 128       # 6272
H0T = 25                      # source tiles in AG table 0
H1T = TILES - H0T             # source tiles in AG table 1
H0 = H0T * 128                # 3200 rows
H1 = H1T * 128                # 3072 rows
NPASS = 3                     # L (local), B (table0), C (table1)
CHUNK_BLOCKS = 8              # gather chunk size (blocks of 128 edges);
                              # >1024 idxs in one dma_gather wedges the
                              # device (2048 and 4096 both verified)
PREFETCH_B = 13               # pass-B chunks emitted right after pass L so
                              # their desc-gen hides under the AG2 window
GL_BUFS = 16                  # pass-L pool: holds ALL L chunks (no reuse)
GB_BUFS = 16                  # pass-B pool: prefetch window + streaming
GC_BUFS = 8                   # pass-C pool: streaming only
OH_CH = 8                     # one-hot columns built per DVE instruction
OH_BUFS = 12                  # one-hot pool depth (ring)
OH_PREBUILD = 6               # leading batches emitted before the tile loop
OUT_GROUP = 8                 # dest tiles per output DMA

# Set by test harness to capture a profile; harmless defaults for grading.
TRACE = False
LAST_RESULTS = None

_F32 = mybir.dt.float32
_BF16 = mybir.dt.bfloat16
_I16 = mybir.dt.int16


def _build_nc(geom):
    """Build + compile the SPMD program.

    geom: dict with
      nb[p]       blocks per pass
      obl[t]      per dest tile: list of (pass, block, oh_col, last_use)
                  where last_use marks the final obligation of that block's
                  chunk (frees the chunk buffer)
      n_oh        total one-hot columns
    """
    nb = geom["nb"]
    obl = geom["obl"]
    n_oh = geom["n_oh"]
    nbatch = (n_oh + OH_CH - 1) // OH_CH
    noh_pad = nbatch * OH_CH
    nb_tot = sum(nb)

    nc = bacc.Bacc("TRN2", target_bir_lowering=False, debug=False,
                   num_devices=NCORES, num_swdge_queues=4,
                   dynamic_dma_scratch_size=32768)

    x_t = nc.dram_tensor("xT", [128, SHARD_PAD], _BF16, kind="ExternalInput")
    w = nc.dram_tensor("w", [128, 128], _BF16, kind="ExternalInput")
    deg_d = nc.dram_tensor("deg", [128, TILES], _F32, kind="ExternalInput")
    iota_d = nc.dram_tensor("iota", [128, 128 * OH_CH], _BF16,
                            kind="ExternalInput")
    idxs_d = nc.dram_tensor("idxs", [128, nb_tot * 8], _I16,
                            kind="ExternalInput")
    segs_d = nc.dram_tensor("segs", [128, noh_pad], _BF16,
                            kind="ExternalInput")
    out_d = nc.dram_tensor("out", [SHARD_PAD, 128], _F32, kind="ExternalOutput")

    # pass-L table: this core's own y, written locally after projection.
    y_local = nc.dram_tensor("y_local", [SHARD_PAD, 128], _BF16)
    # AG inputs (partition-major row order) and gathered tables.
    y_half = [nc.dram_tensor("y_half0", [H0, 128], _BF16),
              nc.dram_tensor("y_half1", [H1, 128], _BF16)]
    table = [nc.dram_tensor("table0", [NCORES * H0, 128], _BF16,
                            addr_space="Shared"),
             nc.dram_tensor("table1", [NCORES * H1, 128], _BF16,
                            addr_space="Shared")]

    pass_blk0 = [0, nb[0], nb[0] + nb[1]]
    n_chunks = [(nb[p] + CHUNK_BLOCKS - 1) // CHUNK_BLOCKS
                for p in range(NPASS)]
    chunk_len = [[min(CHUNK_BLOCKS, nb[p] - ci * CHUNK_BLOCKS)
                  for ci in range(n_chunks[p])] for p in range(NPASS)]

    with tile.TileContext(nc) as tc:
        with (
            tc.tile_pool(name="const", bufs=1) as constp,
            tc.tile_pool(name="psum", bufs=8, space="PSUM") as psp,
            tc.tile_pool(name="oh", bufs=OH_BUFS) as ohp,
            tc.tile_pool(name="gl", bufs=GL_BUFS) as glp,
            tc.tile_pool(name="gb", bufs=GB_BUFS) as gbp,
            tc.tile_pool(name="gc", bufs=GC_BUFS) as gcp,
            tc.tile_pool(name="outp", bufs=3) as outp,
        ):
            # ---- projection inputs first (they gate the critical path) ----
            xt = constp.tile([128, SHARD_PAD], _BF16)
            nc.sync.dma_start(xt[:], x_t[:])
            wt = constp.tile([128, 128], _BF16)
            nc.sync.dma_start(wt[:], w[:])
            deg = constp.tile([128, TILES], _F32)
            nc.sync.dma_start(deg[:], deg_d[:])

            # ---- norm = rsqrt(max(deg, 1)) ----
            dmax = constp.tile([128, TILES], _F32)
            nc.vector.tensor_scalar_max(dmax[:], deg[:], 1.0)
            rcp = constp.tile([128, TILES], _F32)
            nc.vector.reciprocal(rcp[:], dmax[:])
            norm = constp.tile([128, TILES], _F32)
            nc.scalar.sqrt(norm[:], rcp[:])

            # ---- gather-phase index inputs (small; loaded early so DVE
            # one-hot prebuild + pass-L desc-gen start in the dead window) --
            iota = constp.tile([128, 128 * OH_CH], _BF16)
            nc.sync.dma_start(iota[:], iota_d[:])
            segs = constp.tile([128, noh_pad], _BF16)
            nc.sync.dma_start(segs[:], segs_d[:])
            idxs = constp.tile([128, nb_tot * 8], _I16)
            nc.sync.dma_start(idxs[:], idxs_d[:])

            # ---- projection: ybuf[:, t, :] = bf16((X @ W) * norm) ----
            ybuf = constp.tile([128, TILES, 128], _BF16)
            for t in range(TILES):
                ps = psp.tile([128, 128], _F32)
                nc.tensor.matmul(ps[:], xt[:, t * 128:(t + 1) * 128], wt[:],
                                 start=True, stop=True)
                nc.scalar.activation(ybuf[:, t, :], ps[:],
                                     mybir.ActivationFunctionType.Copy,
                                     scale=norm[:, t:t + 1])
                if t == H0T - 1:
                    nc.sync.dma_start(
                        y_half[0].ap().rearrange("(p t) o -> p t o", p=128),
                        ybuf[:, 0:H0T, :])
                    nc.gpsimd.collective_compute(
                        "AllGather", mybir.AluOpType.bypass,
                        replica_groups=[list(range(NCORES))],
                        ins=[y_half[0].ap()], outs=[table[0].ap()])
            # local table write (pass L reads it; no collective involved)
            nc.sync.dma_start(
                y_local.ap().rearrange("(p t) o -> p t o", p=128),
                ybuf[:, :, :])
            nc.sync.dma_start(
                y_half[1].ap().rearrange("(p t) o -> p t o", p=128),
                ybuf[:, H0T:, :])
            nc.gpsimd.collective_compute(
                "AllGather", mybir.AluOpType.bypass,
                replica_groups=[list(range(NCORES))],
                ins=[y_half[1].ap()], outs=[table[1].ap()])

            # ---- one-hot batches: OH_CH columns per DVE op ----
            ohbufs = {}

            def ensure_ohbatch(bi):
                if bi in ohbufs:
                    return
                oh = ohp.tile([128, 128 * OH_CH], _BF16, tag="oh")
                o_v = oh[:].rearrange("p (f j) -> p f j", j=OH_CH)
                i_v = iota[:].rearrange("p (f j) -> p f j", j=OH_CH)
                s_v = (segs[:, bi * OH_CH:(bi + 1) * OH_CH]
                       .unsqueeze(1).broadcast_to([128, 128, OH_CH]))
                nc.vector.tensor_tensor(o_v, i_v, s_v,
                                        mybir.AluOpType.is_equal)
                ohbufs[bi] = oh

            for k in range(OH_PREBUILD):
                ensure_ohbatch(k)

            # ---- gathers: pass L first (no AG dep -> runs in the dead
            # window), then a pass-B prefetch window (hides under AG2),
            # then on-demand in consumption order ----
            gbufs = [[None] * n_chunks[p] for p in range(NPASS)]
            pools = [glp, gbp, gcp]
            srcs = [y_local[:, :], table[0][:, :], table[1][:, :]]
            emit_counter = [0]

            def ensure_chunk(p, ci):
                if gbufs[p][ci] is not None:
                    return
                cl = chunk_len[p][ci]
                gb = pools[p].tile([128, CHUNK_BLOCKS, 128], _BF16,
                                   tag="g%d" % p)
                col0 = (pass_blk0[p] + ci * CHUNK_BLOCKS) * 8
                nc.gpsimd.dma_gather(
                    gb[:, 0:cl, :],
                    srcs[p],
                    idxs[:, col0:col0 + cl * 8],
                    cl * 128,
                    cl * 128,
                    128,
                    queue_num=emit_counter[0] % 4,
                )
                emit_counter[0] += 1
                gbufs[p][ci] = gb

            assert n_chunks[0] <= GL_BUFS, (n_chunks[0], GL_BUFS)
            for ci in range(n_chunks[0]):
                ensure_chunk(0, ci)
            for ci in range(min(PREFETCH_B, n_chunks[1])):
                ensure_chunk(1, ci)

            out_v = out_d.ap().rearrange("(p t) o -> p t o", p=128)
            for t in range(TILES):
                obligations = obl[t]
                ps = psp.tile([128, 128], _F32)
                for k, (p, blk, g, last_use) in enumerate(obligations):
                    ci, slot = divmod(blk, CHUNK_BLOCKS)
                    ensure_chunk(p, ci)
                    bi, jj = divmod(g, OH_CH)
                    ensure_ohbatch(bi)
                    lhsT = (ohbufs[bi][:]
                            .rearrange("p (f j) -> p f j", j=OH_CH)[:, :, jj])
                    nc.tensor.matmul(ps[:], lhsT, gbufs[p][ci][:, slot, :],
                                     start=(k == 0),
                                     stop=(k == len(obligations) - 1))
                    if last_use:
                        gbufs[p][ci] = None
                g0 = (t // OUT_GROUP) * OUT_GROUP
                if t == g0:
                    obuf = outp.tile([128, min(OUT_GROUP, TILES - g0), 128],
                                     _F32)
                nc.scalar.activation(obuf[:, t - g0, :], ps[:],
                                     mybir.ActivationFunctionType.Copy,
                                     scale=norm[:, t:t + 1])
                if t == min(g0 + OUT_GROUP, TILES) - 1:
                    nc.sync.dma_start(out_v[:, g0:t + 1, :], obuf[:])

    nc.compile()
    return nc


def _preprocess(rows, cols):
    """Partition edges into 3 passes with packed slot layout.

    Returns (geom, deg, per_core) where per_core[c] = (idx_tile, seg_tile).
    """
    rows = np.asarray(rows, dtype=np.int64)
    cols = np.asarray(cols, dtype=np.int64)

    deg = np.bincount(rows, minlength=N).astype(np.float32)

    core = rows // SHARD
    t_of = (rows - core * SHARD) >> 7
    seg_of = (rows - core * SHARD) & 127
    src_core = cols // SHARD
    src_l = cols - src_core * SHARD
    src_t = src_l >> 7
    src_p = src_l & 127

    # table index per edge for each possible pass:
    #   L: local y table, partition-major: idx = src_p*TILES + src_t
    #   B: idx = src_core*H0 + src_p*H0T + src_t
    #   C: idx = src_core*H1 + src_p*H1T + (src_t - H0T)
    local = src_core == core
    p_of = np.where(local, 0, np.where(src_t < H0T, 1, 2))
    idx_of = np.where(
        local, src_p * TILES + src_t,
        np.where(src_t < H0T,
                 src_core * H0 + src_p * H0T + src_t,
                 src_core * H1 + src_p * H1T + (src_t - H0T)))

    # per (core, pass, tile) counts -> shared packed layout M/cum
    key = (core * NPASS + p_of) * TILES + t_of
    counts = np.bincount(key, minlength=NCORES * NPASS * TILES).reshape(
        NCORES, NPASS, TILES)
    M = counts.max(axis=0)                      # [NPASS][TILES]
    M[0] = np.maximum(M[0], 1)  # every tile has >=1 slot so PSUM is defined

    nb = []
    cum = []
    for p in range(NPASS):
        c = np.zeros(TILES + 1, np.int64)
        c[1:] = np.cumsum(M[p])
        cum.append(c)
        nb.append(int((c[TILES] + 127) // 128))
    nb_tot = sum(nb)
    pass_blk0 = [0, nb[0], nb[0] + nb[1]]
    pass_slot0 = [0, nb[0] * 128, (nb[0] + nb[1]) * 128]

    # obligations: for each pass/tile, blocks overlapped by its slot range.
    # oh column g numbering: in (tile, pass, block) consumption order.
    obl = [[] for _ in range(TILES)]            # t -> [(p, blk, g, last)]
    n_oh = 0
    chunk_last = {}  # (p, chunk) -> flat position of final obligation
    flat_pos = 0
    obl_meta = []    # (p, t, blk, g, lo, hi) for host seg fill
    for t in range(TILES):
        for p in range(NPASS):
            lo, hi = int(cum[p][t]), int(cum[p][t + 1])
            if hi == lo:
                continue
            b0, b1 = lo // 128, (hi - 1) // 128
            for blk in range(b0, b1 + 1):
                s_lo, s_hi = max(lo, blk * 128), min(hi, (blk + 1) * 128)
                g = n_oh
                n_oh += 1
                obl[t].append([p, blk, g, False])
                obl_meta.append((p, t, blk, g, s_lo, s_hi))
                chunk_last[(p, blk // CHUNK_BLOCKS)] = flat_pos
                flat_pos += 1
    # mark last use of each chunk
    pos = 0
    for t in range(TILES):
        for o in obl[t]:
            if chunk_last[(o[0], o[1] // CHUNK_BLOCKS)] == pos:
                o[3] = True
            pos += 1
    obl = [[tuple(o) for o in obl_t] for obl_t in obl]

    nbatch = (n_oh + OH_CH - 1) // OH_CH
    noh_pad = nbatch * OH_CH

    # shared seg template: seg values depend on slot fills, which differ per
    # core only in WHICH slots are padded (-1); the one-hot column for
    # (p, blk, t) masks other tiles' slots to -1 for every core identically
    # only where the core has no edge.  So segs must be built per core.
    order = np.argsort(key, kind="stable")

    per_core = []
    for c in range(NCORES):
        slot_seg = np.full(nb_tot * 128, -1.0, np.float32)
        slot_idx = np.zeros(nb_tot * 128, np.int16)
        sel = order[(core[order] == c).nonzero()[0]]
        kk = key[sel] - c * NPASS * TILES       # p*TILES + t, sorted
        p_sel = kk // TILES
        t_sel = kk % TILES
        grp_start = np.searchsorted(kk, np.arange(NPASS * TILES), side="left")
        pos_in_grp = np.arange(sel.size) - grp_start[kk]
        base = (np.take(pass_slot0, p_sel)
                + cum[0][t_sel] * (p_sel == 0)
                + cum[1][t_sel] * (p_sel == 1)
                + cum[2][t_sel] * (p_sel == 2))
        dst = base + pos_in_grp
        slot_idx[dst] = idx_of[sel].astype(np.int16)
        slot_seg[dst] = seg_of[sel].astype(np.float32)

        # seg columns per obligation: slice [s_lo, s_hi) of the block,
        # other slots -1.
        seg_cols = np.full((noh_pad, 128), -1.0, np.float32)
        for (p, t, blk, g, s_lo, s_hi) in obl_meta:
            sb = pass_slot0[p] + blk * 128
            a = s_lo - blk * 128
            b = s_hi - blk * 128
            seg_cols[g, a:b] = slot_seg[sb + a: sb + b]

        idx_tile = np.tile(slot_idx.reshape(nb_tot * 8, 16).T, (8, 1))
        seg_tile = np.ascontiguousarray(seg_cols.T).astype(ml_dtypes.bfloat16)
        per_core.append((idx_tile, seg_tile))

    geom = {"nb": nb, "obl": obl, "n_oh": n_oh}
    return geom, deg, per_core


def kernel(drug_f, disease_f, drug_w, disease_w, rows, cols):
    global LAST_RESULTS
    drug_f = np.asarray(drug_f, np.float32)
    disease_f = np.asarray(disease_f, np.float32)
    drug_w = np.asarray(drug_w, np.float32)
    disease_w = np.asarray(disease_w, np.float32)

    geom, deg, per_core = _preprocess(rows, cols)
    nc = _build_nc(geom)

    feats = np.concatenate([drug_f, disease_f], axis=0)  # [N, 128]
    # iota[p, f*OH_CH + j] = f
    iota = np.repeat(np.arange(128, dtype=np.float32), OH_CH)
    iota = np.tile(iota[None, :], (128, 1)).astype(ml_dtypes.bfloat16)

    in_maps = []
    for c in range(NCORES):
        sh = feats[c * SHARD:(c + 1) * SHARD]           # [6250, 128]
        x_t = np.zeros((128, SHARD_PAD), ml_dtypes.bfloat16)
        x_t[:, :SHARD] = sh.T.astype(ml_dtypes.bfloat16)
        dg = np.ones((SHARD_PAD,), np.float32)
        dg[:SHARD] = deg[c * SHARD:(c + 1) * SHARD]
        idx_tile, seg_tile = per_core[c]
        in_maps.append({
            "xT": x_t,
            "w": (drug_w if c < 4 else disease_w).astype(ml_dtypes.bfloat16),
            "deg": dg.reshape(TILES, 128).T.copy(),
            "iota": iota,
            "idxs": idx_tile,
            "segs": seg_tile,
        })

    res = run_bass_kernel_spmd(nc, in_maps, core_ids=list(range(NCORES)),
                               trace=TRACE)
    LAST_RESULTS = res

    # out rows are partition-major (p*TILES + t); restore node order.
    outs = []
    for c in range(NCORES):
        r = res.results[c]["out"].reshape(128, TILES, 128)
        outs.append(r.transpose(1, 0, 2).reshape(SHARD_PAD, 128)[:SHARD])
    return np.concatenate(outs, axis=0)
